# revision 1
# baseline (speedup 1.0000x reference)
"""MultiBoxLoss (RetinaFace-style) Trainium2 Bass kernel — v2.

Per core (R=8 rows, priors padded to PP=16896 = 128*132):
  Phase 1: partitions = (rsub,g) pairs (2 chunks of 128), free = priors.
    fp16 phi = inter/(areaA+areaB) via tensor_scalar ops against per-partition
    gt scalars; relu/reciprocal on ACT; area-sum on Pool; fused
    tensor_tensor_reduce stores phi and accumulates the per-gt best-prior max.
    Forced matches: best prior of each valid gt bumped to 2+g/256 by a second
    diag-scaled PE matmul accumulated into the transpose PSUM.
  Phase 2: x-layout [prior%128, block, r, g] after PE transposes; btov = max_g,
    argmax via is_ge + (31-g) max-encode; pos mask; gather indices 32r+btidx.
  Gather: one gpsimd indirect_copy of the 16-wide fp16 per-(r,gt) table using
    DRAM-bounced wrapped uint16 indices; PE back-transpose to x-layout.
  Losses: encode = (gathered - A)*B with host-prepped patterns; smooth-L1 via
    fused TTR square-accumulations; CE via ACT Softplus; top-k negatives via
    threshold bisection (count on DVE, combine via partition_all_reduce) with
    the exact t*(k-cnt) correction.

Host side prepares O(P + B*G) marshalling tensors (fp16 casts, prior corner/
area vectors, encode A/B patterns, per-gt table); all O(B*P*G) work on device.
"""

import numpy as np

P_REAL = 16800
PP = 16896
NB = 132
G = 32
R = 8
NCORES = 8
F = 1056
NFC = PP // F        # 16
TH_POS = float(np.float32(7.0 / 27.0))
TH_VALID = float(np.float32(1.0 / 6.0))
NEGPOS = 7
NSEARCH = 10
EVB = 12             # 132 = 11 * 12 transpose blocks per PSUM batch

_cached = {}


def _build_module(debug=False, stage=99):
    import concourse.bacc as bacc
    import concourse.tile as tile
    import concourse.mybir as mybir

    dt = mybir.dt
    nc = bacc.Bacc("TRN2", target_bir_lowering=False, debug=False,
                   enable_asserts=False, num_devices=NCORES)

    ext = dict(kind="ExternalInput")
    tens = {
        "clsX": nc.dram_tensor("clsX", [128, R * NB * 2], dt.float16, **ext).ap(),
        "datX": nc.dram_tensor("datX", [128, R * NB * 14], dt.float16, **ext).ap(),
        "pvec": nc.dram_tensor("pvec", [5, PP], dt.float16, **ext).ap(),
        "gtsc": nc.dram_tensor("gtsc", [2, 128, 8], dt.float32, **ext).ap(),
        "tab": nc.dram_tensor("tab", [16, 256], dt.float32, **ext).ap(),
        "abX": nc.dram_tensor("abX", [128, 2 * NB * 16], dt.float16, **ext).ap(),
        "out": nc.dram_tensor("out", [1, 16], dt.float32, kind="ExternalOutput").ap(),
        "wdram": nc.dram_tensor("wdram", [16, NB * 8, R], dt.uint16, kind="Internal").ap(),
        "bpdram": nc.dram_tensor("bpdram", [2, 128], dt.float32, kind="Internal").ap(),
    }

    dbg = {}
    if debug:
        mk = lambda n, sh, d: nc.dram_tensor(n, sh, d, kind="ExternalOutput").ap()
        dbg["tphi"] = mk("dbg_tphi", [128, NB * 256], dt.float16)
        dbg["btov"] = mk("dbg_btov", [128, NB * 8], dt.float16)
        dbg["widx"] = mk("dbg_widx", [128, NB * 8], dt.uint16)
        dbg["gt"] = mk("dbg_gt", [128, NB * 128], dt.float16)
        dbg["pos"] = mk("dbg_pos", [128, NB * 8], dt.float16)
        dbg["lossc"] = mk("dbg_lossc", [128, NB * 8], dt.float16)

    with tile.TileContext(nc) as tc:
        _body(tc, nc, tens, dbg, stage)
    nc.compile()
    return nc


def _body(tc, nc, T, dbg, stage=99):
    import concourse.mybir as mybir
    from concourse import bass_isa as BI
    from contextlib import ExitStack

    dt = mybir.dt
    A = mybir.AluOpType
    AF = mybir.ActivationFunctionType
    AX = mybir.AxisListType
    f16 = dt.float16
    f32 = dt.float32
    V = nc.vector
    S = nc.scalar
    Q = nc.gpsimd

    def ts(out, in0, s1, op0, s2=None, op1=None, accum=None):
        V.tensor_scalar(out, in0, s1, s2, op0=op0,
                        **({"op1": op1} if op1 is not None else {}),
                        **({"accum_out": accum} if accum is not None else {}))

    ctx = ExitStack()
    glob = ctx.enter_context(tc.tile_pool(name="glob", bufs=1))

    def _bail():
        with tc.tile_pool(name="bail", bufs=1) as bp:
            z = bp.tile([1, 16], f32, tag="z", name="z")
            V.memset(z, 1.0)
            Q.dma_start(T["out"], z)
        ctx.close()

    # ---------- small persistent tiles ----------
    invg = glob.tile([128, G], f16, tag="invg", name="invg")
    with tc.tile_pool(name="prep2", bufs=1) as prep2:
        invg_i = prep2.tile([128, G], dt.int32, tag="invgi", name="invgi")
        Q.iota(invg_i, pattern=[[-1, G]], base=G - 1, channel_multiplier=0)
        invgf = prep2.tile([128, G], f32, tag="invgf", name="invgf")
        V.tensor_copy(invgf, invg_i)
        V.tensor_copy(invg, invgf)                       # 31-g

    r32 = glob.tile([128, R], f16, tag="r32", name="r32")
    for r in range(R):
        V.memset(r32[:, r:r + 1], float(32 * r))

    acc = glob.tile([128, 16], f32, tag="acc", name="acc")
    V.memset(acc, 0.0)

    ph2 = ctx.enter_context(tc.tile_pool(name="ph2", bufs=1))
    btov = ph2.tile([128, NB, 8], f16, tag="btov", name="btov")
    pos = ph2.tile([128, NB, 8], f16, tag="pos", name="pos")
    wx = ph2.tile([128, NB, 8], dt.uint16, tag="wx", name="wx")
    anyv = ph2.tile([128, 8], f32, tag="anyv", name="anyv")
    npr = ph2.tile([128, 8], f32, tag="npr", name="npr")
    nprall = ph2.tile([128, 8], f32, tag="nprall", name="nprall")
    kk = ph2.tile([128, 8], f32, tag="kk", name="kk")

    tphip = tc.tile_pool(name="tphip", bufs=1)
    tphi_pool = tphip.__enter__()
    t_phi = tphi_pool.tile([128, NB, 256], f16, tag="tphi", name="tphi")

    # ================= phase 1 =================
    FB = 2112
    NFB = PP // FB
    with tc.tile_pool(name="ph1", bufs=1) as ph1, \
         tc.tile_pool(name="pvp", bufs=2) as pvp, \
         tc.tile_pool(name="wkp", bufs=2) as wkp:

        gts = []
        for c in range(2):
            g_t = ph1.tile([128, 8], f32, tag=f"gts{c}", name=f"gts{c}")
            nc.sync.dma_start(g_t, T["gtsc"].rearrange("c q v -> c q v")[c:c + 1]
                              .rearrange("o q v -> (o q) v"))
            gts.append(g_t)

        pvsrc = T["pvec"].rearrange("q (o p) -> o q p", o=1)
        for c in range(2):
            g_t = gts[c]
            phi = ph1.tile([128, PP], f16, tag="phi", name=f"phi{c}")
            for fc in range(NFB):
                fs = slice(fc * FB, (fc + 1) * FB)
                pv = pvp.tile([128, 5, FB], f16, tag="pv", name="pv")
                nc.sync.dma_start(pv, pvsrc[:, :, fs].broadcast_to([128, 5, FB]))
                ltx = wkp.tile([128, FB], f16, tag="w0", name="ltx")
                t0 = wkp.tile([128, FB], f16, tag="w1", name="t0")
                dx = wkp.tile([128, FB], f16, tag="w2", name="dx")
                lty = wkp.tile([128, FB], f16, tag="w0", name="lty")
                t1 = wkp.tile([128, FB], f16, tag="w1", name="t1")
                dy = wkp.tile([128, FB], f16, tag="w3", name="dy")
                Iv = wkp.tile([128, FB], f16, tag="w4", name="Iv")
                Sv = wkp.tile([128, FB], f16, tag="w3", name="Sv")
                rr = wkp.tile([128, FB], f16, tag="w2", name="rr")
                ts(Sv, pv[:, 4], g_t[:, 4:5], A.add)
                S.activation(rr, Sv, AF.Ln)
                S.activation(rr, rr, AF.Exp, scale=-1.0)
                ts(ltx, pv[:, 0], g_t[:, 0:1], A.max)
                ts(t0, pv[:, 1], g_t[:, 1:2], A.min)
                V.tensor_tensor(dx, t0, ltx, op=A.subtract)
                ts(lty, pv[:, 2], g_t[:, 2:3], A.max)
                ts(t1, pv[:, 3], g_t[:, 3:4], A.min)
                V.tensor_tensor(dy, t1, lty, op=A.subtract)
                S.activation(dx, dx, AF.Relu)
                ts(dy, dy, 0.0, A.max)
                V.tensor_tensor(Iv, dx, dy, op=A.mult)
                V.tensor_tensor(phi[:, fs], Iv, rr, op=A.mult)
                if fc == 0:
                    rmax = ph1.tile([128, FB], f16, tag="rmax", name=f"rmax{c}")
                    V.tensor_copy(rmax, phi[:, fs])
                else:
                    V.tensor_tensor(rmax, rmax, phi[:, fs], op=A.max)

            bp = ph1.tile([128, 1], f32, tag="bp", name=f"bp{c}")
            V.tensor_reduce(bp, rmax, axis=AX.X, op=A.max)
            vm = ph1.tile([128, 1], f32, tag="vm", name=f"vm{c}")
            gate = ph1.tile([128, 1], f32, tag="gate", name=f"gate{c}")
            t2 = ph1.tile([128, 1], f32, tag="t2", name=f"t2{c}")
            ts(vm, bp, TH_VALID, A.is_ge)
            nc.sync.dma_start(T["bpdram"][c:c + 1].rearrange("o q -> q o"), vm)
            ts(gate, bp, float(1.0 - 2.0 ** -10), A.mult)
            ts(t2, vm, -1e9, A.mult, 1e9, A.add)
            V.tensor_tensor(gate, gate, t2, op=A.add)
            if stage >= 2:
                for fc in range(NFB):
                    fs = slice(fc * FB, (fc + 1) * FB)
                    eqc = wkp.tile([128, FB], f16, tag="w4", name="eqc")
                    ts(eqc, phi[:, fs], gate, A.is_ge, gts[c][:, 5:6], A.mult)
                    V.tensor_tensor(phi[:, fs], phi[:, fs], eqc, op=A.max)
                S.dma_start_transpose(
                    t_phi.rearrange("p b q -> p b q")[:, :, c * 128:(c + 1) * 128],
                    phi)

    if stage <= 2:
        tphip.__exit__(None, None, None)
        _bail()
        return
    if dbg:
        Q.dma_start(dbg["tphi"], t_phi.rearrange("p b q -> p (b q)"))

    # ================= phase 2 =================
    t4 = t_phi.rearrange("p b (r g) -> p b r g", g=G)
    with tc.tile_pool(name="trp", bufs=1) as trp:
        m16 = trp.tile([128, NB, 8, 16], f16, tag="m16", name="m16")
        V.tensor_tensor(m16, t4[:, :, :, 0:16], t4[:, :, :, 16:32], op=A.max)
        V.tensor_tensor(m16[:, :, :, 0:8], m16[:, :, :, 0:8],
                        m16[:, :, :, 8:16], op=A.max)
        V.tensor_tensor(m16[:, :, :, 0:4], m16[:, :, :, 0:4],
                        m16[:, :, :, 4:8], op=A.max)
        V.tensor_tensor(m16[:, :, :, 0:2], m16[:, :, :, 0:2],
                        m16[:, :, :, 2:4], op=A.max)
        V.tensor_tensor(btov, m16[:, :, :, 0], m16[:, :, :, 1], op=A.max)

    with tc.tile_pool(name="anyp", bufs=1) as anyp:
        vrow = anyp.tile([1, 256], f32, tag="vrow", name="vrow")
        nc.sync.dma_start(vrow, T["bpdram"].rearrange("(o c) q -> o (c q)", o=1))
        any1 = anyp.tile([1, 8], f32, tag="any1", name="any1")
        V.tensor_reduce(any1, vrow.rearrange("o (r g) -> o r g", g=G),
                        axis=AX.X, op=A.max)
        Q.partition_broadcast(anyv, any1, channels=128)

    ts(pos, btov, TH_POS, A.is_ge)
    anyv_bc = anyv.unsqueeze(1).broadcast_to([128, NB, 8])
    V.tensor_tensor(pos, pos, anyv_bc, op=A.mult)

    with tc.tile_pool(name="argp", bufs=1) as argp:
        btov2 = argp.tile([128, NB, 8, 2], f16, tag="btov2", name="btov2")
        bv2 = btov.unsqueeze(3).broadcast_to([128, NB, 8, 2])
        V.tensor_copy(btov2, bv2)
        eq2 = argp.tile([128, NB, 8, G], f16, tag="eq2", name="eq2")
        btov_bc = btov2.rearrange("p b r t -> p (b r) t").unsqueeze(2) \
                       .broadcast_to([128, NB * 8, 16, 2])
        t5 = t_phi.rearrange("p b (r h t) -> p (b r) h t", r=8, h=16)
        e5 = eq2.rearrange("p b r (h t) -> p (b r) h t", h=16)
        V.tensor_tensor(e5, t5, btov_bc, op=A.is_ge)
        invg_bc = invg.unsqueeze(1).unsqueeze(1).broadcast_to([128, NB, 8, G])
        V.tensor_tensor(eq2, eq2, invg_bc, op=A.mult)
        e4 = eq2
        V.tensor_tensor(e4[:, :, :, 0:16], e4[:, :, :, 0:16],
                        e4[:, :, :, 16:32], op=A.max)
        V.tensor_tensor(e4[:, :, :, 0:8], e4[:, :, :, 0:8],
                        e4[:, :, :, 8:16], op=A.max)
        V.tensor_tensor(e4[:, :, :, 0:4], e4[:, :, :, 0:4],
                        e4[:, :, :, 4:8], op=A.max)
        V.tensor_tensor(e4[:, :, :, 0:2], e4[:, :, :, 0:2],
                        e4[:, :, :, 2:4], op=A.max)
        btenc = argp.tile([128, NB, 8], f16, tag="btenc", name="btenc")
        V.tensor_tensor(btenc, e4[:, :, :, 0], e4[:, :, :, 1], op=A.max)
        ts(btenc, btenc, -1.0, A.mult, float(G - 1), A.add)   # btidx
        r32_bc = r32.unsqueeze(1).broadcast_to([128, NB, 8])
        V.tensor_tensor(btenc, btenc, r32_bc, op=A.add)       # 32r + btidx
        V.tensor_copy(wx, btenc)
    tphip.__exit__(None, None, None)

    V.tensor_reduce(npr, pos.rearrange("p b r -> p r b"), axis=AX.X, op=A.add)
    Q.partition_all_reduce(nprall, npr, channels=128, reduce_op=BI.ReduceOp.add)
    ts(kk, nprall, float(NEGPOS), A.mult, float(P_REAL - 1), A.min)

    if stage <= 3:
        _bail()
        return
    # ================= CE =================
    cep = ctx.enter_context(tc.tile_pool(name="cep", bufs=1))
    lossc = cep.tile([128, NB, 8], f16, tag="lossc", name="lossc")
    with tc.tile_pool(name="cew", bufs=1) as cew:
        clst = cew.tile([128, 8, NB, 2], f16, tag="clst", name="clst")
        nc.sync.dma_start(clst.rearrange("p r b c -> p (r b c)"), T["clsX"])
        clv = clst.rearrange("p r b c -> p b r c")
        dce = cew.tile([128, NB, 8], f16, tag="dce", name="dce")
        V.tensor_tensor(dce, clv[:, :, :, 1], clv[:, :, :, 0], op=A.subtract)
        sp = cew.tile([128, NB, 8], f16, tag="sp", name="sp")
        S.activation(sp.rearrange("p b r -> p (b r)"),
                     dce.rearrange("p b r -> p (b r)"), AF.Abs)
        S.activation(sp.rearrange("p b r -> p (b r)"),
                     sp.rearrange("p b r -> p (b r)"), AF.Exp, scale=-1.0)
        S.activation(sp.rearrange("p b r -> p (b r)"),
                     sp.rearrange("p b r -> p (b r)"), AF.Ln, bias=1.0)
        reld = cew.tile([128, NB, 8], f16, tag="reld", name="reld")
        ts(reld, dce, 0.0, A.max)
        cp1 = cew.tile([128, NB, 8], f16, tag="cp1", name="cp1")
        V.tensor_tensor(cp1, pos, dce, op=A.mult)
        V.tensor_tensor(cp1, reld, cp1, op=A.subtract)
        ce = cew.tile([128, NB, 8], f16, tag="ce", name="ce")
        V.tensor_tensor(ce, cp1, sp, op=A.add)
        npos_t = cew.tile([128, NB, 8], f16, tag="npos_t", name="npos_t")
        ts(npos_t, pos, -1.0, A.mult, 1.0, A.add)
        V.tensor_tensor(lossc, ce, npos_t, op=A.mult)
        S.activation(ce.rearrange("p b r -> p (b r)"),
                     ce.rearrange("p b r -> p (b r)"), AF.Copy,
                     accum_out=acc[:, 13:14])
        S.activation(lossc.rearrange("p b r -> p (b r)"),
                     lossc.rearrange("p b r -> p (b r)"), AF.Copy,
                     accum_out=acc[:, 14:15])
    if dbg:
        Q.dma_start(dbg["lossc"], lossc.rearrange("p b r -> p (b r)"))


    # ---- W bounce + table + gather ----
    wview = T["wdram"].rearrange("m (b p8) r -> p8 m b r", p8=8)
    for p8 in range(8):
        nc.sync.dma_start(wview[p8:p8 + 1].rearrange("o m b r -> (o m) b r"),
                          wx[16 * p8:16 * (p8 + 1)])
    g_tt = ctx.enter_context(tc.tile_pool(name="gtt", bufs=1))
    g_t = g_tt.tile([128, NB, 128], f16, tag="gt", name="gt")
    with tc.tile_pool(name="gath", bufs=1) as gath:
        wwrap = gath.tile([128, NB * 8], dt.uint16, tag="wwrap", name="wwrap")
        wl = T["wdram"].rearrange("m s r -> r m s")
        for r in range(R):
            for h in range(2):
                hs = slice(h * 528, (h + 1) * 528)
                nc.sync.dma_start(wwrap[16 * r:16 * (r + 1), hs],
                                  wl[r:r + 1].rearrange("o m s -> (o m) s")[:, hs])
        dtab = gath.tile([128, 256], f32, tag="dtab", name="dtab")
        for r in range(R):
            nc.sync.dma_start(dtab[16 * r:16 * r + 16, :], T["tab"])
        g_out = gath.tile([128, PP, 1], f32, tag="gout", name="gout")
        g16 = gath.tile([128, PP], f16, tag="g16", name="g16")
        gflat = g_out.rearrange("p x o -> p (x o)")
        off = 0
        while off < PP:
            nidx = min(1024, PP - off)
            Q.indirect_copy(g_out[:, off:off + nidx, :], dtab,
                            wwrap[:, off // 16:(off + nidx) // 16], True)
            S.activation(g16[:, off:off + nidx], gflat[:, off:off + nidx],
                         AF.Copy)
            S.dma_start_transpose(g_t[:, off // 128:(off + nidx) // 128, :],
                                  g16[:, off:off + nidx])
            off += nidx

    if dbg:
        Q.dma_start(dbg["btov"], btov.rearrange("p b r -> p (b r)"))
        Q.dma_start(dbg["widx"], wx.rearrange("p b r -> p (b r)"))
        Q.dma_start(dbg["pos"], pos.rearrange("p b r -> p (b r)"))

    if dbg:
        Q.dma_start(dbg["gt"], g_t.rearrange("p b q -> p (b q)"))

    if stage <= 4:
        _bail()
        return
    # ================= losses =================
    lossp = ctx.enter_context(tc.tile_pool(name="lossp", bufs=1))
    g4 = g_t.rearrange("p b (r c) -> p b r c", c=16)
    pos1 = lossp.tile([128, NB, 8], f16, tag="pos1", name="pos1")
    V.tensor_tensor(pos1, pos, g4[:, :, :, 14], op=A.mult)
    S.activation(pos1.rearrange("p b r -> p (b r)"),
                 pos1.rearrange("p b r -> p (b r)"), AF.Copy,
                 accum_out=acc[:, 12:13])             # npos1 partial

    with tc.tile_pool(name="encp", bufs=1) as encp:
        av = encp.tile([128, NB, 16], f16, tag="av", name="av")
        bv = encp.tile([128, NB, 16], f16, tag="bv", name="bv")
        absrc = T["abX"].rearrange("p (t x) -> p t x", t=2)
        nc.sync.dma_start(av.rearrange("p b c -> p (b c)"), absrc[:, 0])
        nc.sync.dma_start(bv.rearrange("p b c -> p (b c)"), absrc[:, 1])
        ddat = encp.tile([128, 8, NB, 14], f16, tag="ddat", name="ddat")
        nc.sync.dma_start(ddat.rearrange("p r b c -> p (r b c)"), T["datX"])
        # encode in place over the gathered columns, then diff/mask in place
        a_bc = av.unsqueeze(2).broadcast_to([128, NB, 8, 16])
        b_bc = bv.unsqueeze(2).broadcast_to([128, NB, 8, 16])
        dv = ddat.rearrange("p r b c -> p b r c")
        junk = lossp.tile([128, 8, NB, 5], f32, tag="junk", name="junk")
        pos_bc4 = pos.unsqueeze(3).broadcast_to([128, NB, 8, 4])
        pos1_bc = pos1.unsqueeze(3).broadcast_to([128, NB, 8, 10])
        for hb in range(2):
            bs = slice(hb * 66, (hb + 1) * 66)
            V.tensor_tensor(g4[:, bs], g4[:, bs], a_bc[:, bs], op=A.subtract)
            V.tensor_tensor(g4[:, bs], g4[:, bs], b_bc[:, bs], op=A.mult)
            V.tensor_tensor(dv[:, bs, :, :], g4[:, bs, :, 0:14],
                            dv[:, bs, :, :], op=A.subtract)
            V.tensor_tensor(dv[:, bs, :, 0:4], dv[:, bs, :, 0:4],
                            pos_bc4[:, bs], op=A.mult)
            V.tensor_tensor(dv[:, bs, :, 4:14], dv[:, bs, :, 4:14],
                            pos1_bc[:, bs], op=A.mult)

        au = encp.tile([128, 8, NB, 5], f16, tag="au", name="au")
        for h in range(2):
            rs = slice(h * 4, (h + 1) * 4)
            for (sl, w, c0) in ((slice(0, 4), 4, 0), (slice(4, 9), 5, 2),
                                (slice(9, 14), 5, 4)):
                cc = c0 + 6 * h
                dm2 = ddat[:, rs, :, sl]
                jj = junk[:, rs, :, 0:w]
                S.activation(jj, dm2, AF.Square, accum_out=acc[:, cc:cc + 1])
                auw = au[:, rs, :, 0:w]
                S.activation(auw, dm2, AF.Abs)
                ts(auw, auw, 1.0, A.subtract, 0.0, A.max)
                S.activation(jj, auw, AF.Square,
                             accum_out=acc[:, cc + 1:cc + 2])

    if stage <= 5:
        _bail()
        return
    if stage <= 6:
        _bail()
        return
    # ================= top-k bisection =================
    bis = ctx.enter_context(tc.tile_pool(name="bis", bufs=1))
    lo = bis.tile([128, 8], f32, tag="lo", name="lo")
    hi = bis.tile([128, 8], f32, tag="hi", name="hi")
    tm = bis.tile([128, 8], f32, tag="tm", name="tm")
    cnt = bis.tile([128, 8], f32, tag="cnt", name="cnt")
    cntall = bis.tile([128, 8], f32, tag="cntall", name="cntall")
    gtm = bis.tile([128, 8], dt.uint8, tag="gtm", name="gtm")
    ivm = bis.tile([128, 8], dt.uint8, tag="ivm", name="ivm")
    cg = bis.tile([128, NB, 8], f16, tag="cg", name="cg")
    tm16 = bis.tile([128, 8], f16, tag="tm16", name="tm16")
    V.memset(lo, 0.0)
    V.memset(hi, 64.0)
    tm_bc = tm16.unsqueeze(1).broadcast_to([128, NB, 8])
    for it in range(NSEARCH + 1):
        V.tensor_tensor(tm, lo, hi, op=A.add)
        ts(tm, tm, 0.5, A.mult)
        V.tensor_copy(tm16, tm)
        V.tensor_tensor(cg, lossc, tm_bc, op=A.is_gt)
        V.tensor_reduce(cnt, cg.rearrange("p b r -> p r b"), axis=AX.X, op=A.add)
        Q.partition_all_reduce(cntall, cnt, channels=128,
                               reduce_op=BI.ReduceOp.add)
        if it < NSEARCH:
            V.tensor_tensor(gtm, cntall, kk, op=A.is_gt)
            ts(ivm, gtm, 0.0, A.is_equal)
            V.copy_predicated(lo, gtm, tm)
            V.copy_predicated(hi, ivm, tm)
    mg = bis.tile([128, NB, 8], f16, tag="mg", name="mg")
    V.tensor_tensor(mg, lossc, cg, op=A.mult)
    S.activation(mg.rearrange("p b r -> p (b r)"),
                 mg.rearrange("p b r -> p (b r)"), AF.Copy,
                 accum_out=acc[:, 15:16])

    # ================= final pack =================
    accall = bis.tile([128, 16], f32, tag="accall", name="accall")
    Q.partition_all_reduce(accall, acc, channels=128, reduce_op=BI.ReduceOp.add)
    corr = bis.tile([128, 8], f32, tag="corr", name="corr")
    V.tensor_tensor(corr, kk, cntall, op=A.subtract)
    V.tensor_tensor(corr, corr, tm, op=A.mult)
    outsb = bis.tile([128, 16], f32, tag="outsb", name="outsb")
    V.memset(outsb, 0.0)
    # fold the two half-accumulator banks together: col k += col k+6
    V.tensor_tensor(accall[:, 0:6], accall[:, 0:6], accall[:, 6:12], op=A.add)
    # loss_l = 0.5*(locsq - locrsq)
    V.tensor_tensor(outsb[:, 0:1], accall[:, 0:1], accall[:, 1:2], op=A.subtract)
    ts(outsb[:, 0:1], outsb[:, 0:1], 0.5, A.mult)
    # conf = (ce_sum - lossc_sum) + negsum + corr_sum
    csum = bis.tile([128, 1], f32, tag="csum", name="csum")
    V.tensor_reduce(csum, corr, axis=AX.X, op=A.add)
    V.tensor_tensor(outsb[:, 1:2], accall[:, 13:14], accall[:, 14:15], op=A.subtract)
    V.tensor_tensor(outsb[:, 1:2], outsb[:, 1:2], accall[:, 15:16], op=A.add)
    V.tensor_tensor(outsb[:, 1:2], outsb[:, 1:2], csum, op=A.add)
    # landm = 0.5*((lmAsq+lmBsq) - (lmArsq+lmBrsq))
    lmt = bis.tile([128, 2], f32, tag="lmt", name="lmt")
    V.tensor_tensor(lmt[:, 0:1], accall[:, 2:3], accall[:, 4:5], op=A.add)
    V.tensor_tensor(lmt[:, 1:2], accall[:, 3:4], accall[:, 5:6], op=A.add)
    V.tensor_tensor(outsb[:, 2:3], lmt[:, 0:1], lmt[:, 1:2], op=A.subtract)
    ts(outsb[:, 2:3], outsb[:, 2:3], 0.5, A.mult)
    V.tensor_reduce(outsb[:, 3:4], nprall, axis=AX.X, op=A.add)    # num_pos
    V.tensor_copy(outsb[:, 4:5], accall[:, 12:13])                 # num_pos1
    Q.dma_start(T["out"], outsb[0:1, :])

    ctx.close()


# ---------------------------------------------------------------------------
def _host_prep(inputs):
    f16d = np.float16
    f32d = np.float32
    cls_data = np.asarray(inputs["cls_data"], f32d)
    loc_data = np.asarray(inputs["loc_data"], f32d)
    landm_data = np.asarray(inputs["landm_data"], f32d)
    priors = np.asarray(inputs["priors"], f32d)
    targets = np.asarray(inputs["targets"], f32d)
    B = cls_data.shape[0]
    pad = PP - P_REAL

    clsp = np.concatenate(
        [cls_data, np.tile(np.array([0.0, -60.0], f32d), (B, pad, 1))], 1).astype(f16d)
    locp = np.concatenate([loc_data, np.zeros((B, pad, 4), f32d)], 1).astype(f16d)
    lmdp = np.concatenate([landm_data, np.zeros((B, pad, 10), f32d)], 1).astype(f16d)
    pp = np.concatenate(
        [priors, np.tile(np.array([3.0, 3.0, 1.0, 1.0], f32d), (pad, 1))], 0)

    # x-layout device views: [...] indexed [p0, r, b, c] with prior = 128*b + p0
    dat_all = np.concatenate([locp, lmdp], axis=2)          # [B, PP, 14]
    dat_all = dat_all.reshape(B, NB, 128, 14).transpose(2, 0, 1, 3)  # [128,B,NB,14]
    cls_all = clsp.reshape(B, NB, 128, 2).transpose(2, 0, 1, 3)      # [128,B,NB,2]

    pcx, pcy, pw, ph = pp[:, 0], pp[:, 1], pp[:, 2], pp[:, 3]
    pvec = np.stack([pcx - pw / 2, pcx + pw / 2, pcy - ph / 2, pcy + ph / 2,
                     pw * ph]).astype(f16d)

    i01pw = f32d(1.0) / (f32d(0.1) * pw)
    i01ph = f32d(1.0) / (f32d(0.1) * ph)
    logpw = np.log(pw, dtype=f32d)
    logph = np.log(ph, dtype=f32d)
    Ap = np.zeros((PP, 16), f32d)
    Bp = np.zeros((PP, 16), f32d)
    Ap[:, 0] = pcx; Ap[:, 1] = pcy; Ap[:, 2] = logpw; Ap[:, 3] = logph
    Bp[:, 0] = i01pw; Bp[:, 1] = i01ph; Bp[:, 2] = 5.0; Bp[:, 3] = 5.0
    for k in range(5):
        Ap[:, 4 + 2 * k] = pcx; Ap[:, 5 + 2 * k] = pcy
        Bp[:, 4 + 2 * k] = i01pw; Bp[:, 5 + 2 * k] = i01ph
    abX = np.ascontiguousarray(
        np.stack([Ap.astype(f16d), Bp.astype(f16d)])     # [2, PP, 16]
        .reshape(2, NB, 128, 16).transpose(2, 0, 1, 3)   # [128, 2, NB, 16]
        .reshape(128, 2 * NB * 16))

    gx1 = targets[:, :, 0]; gy1 = targets[:, :, 1]
    gx2 = targets[:, :, 2]; gy2 = targets[:, :, 3]
    garea = (gx2 - gx1) * (gy2 - gy1)
    lab = targets[:, :, 14]
    gvals = np.arange(G, dtype=f32d)

    Tt = np.zeros((B, G, 16), f32d)
    Tt[:, :, 0] = (gx1 + gx2) * 0.5
    Tt[:, :, 1] = (gy1 + gy2) * 0.5
    Tt[:, :, 2] = np.log(gx2 - gx1)
    Tt[:, :, 3] = np.log(gy2 - gy1)
    Tt[:, :, 4:14] = targets[:, :, 4:14]
    Tt[:, :, 14] = (lab > 0).astype(f32d)
    T16 = Tt.astype(f16d)

    in_maps = []
    for c in range(NCORES):
        s = slice(c * R, (c + 1) * R)
        gtsc = np.zeros((2, 128, 8), f32d)
        for ch in range(2):
            for rsub in range(4):
                r = 4 * ch + rsub
                qs = slice(rsub * 32, rsub * 32 + 32)
                gtsc[ch, qs, 0] = gx1[c * R + r]
                gtsc[ch, qs, 1] = gx2[c * R + r]
                gtsc[ch, qs, 2] = gy1[c * R + r]
                gtsc[ch, qs, 3] = gy2[c * R + r]
                gtsc[ch, qs, 4] = garea[c * R + r]
                gtsc[ch, qs, 5] = 2.0 + gvals / 256.0
        tabc = np.zeros((16, 256), f32d)
        for r in range(R):
            tabc[:, 32 * r:32 * r + 32] = T16[c * R + r].T.astype(f32d)
        in_maps.append({
            "clsX": np.ascontiguousarray(cls_all[:, s]).reshape(128, R * NB * 2),
            "datX": np.ascontiguousarray(dat_all[:, s]).reshape(128, R * NB * 14),
            "pvec": pvec,
            "gtsc": gtsc,
            "tab": tabc,
            "abX": abX,
        })
    return in_maps


def _combine(parts):
    tot = np.zeros(16, np.float64)
    for p in parts:
        tot += p["out"].reshape(-1).astype(np.float64)
    N = max(tot[3], 1.0)
    N1 = max(tot[4], 1.0)
    return np.array([tot[0] / N, tot[1] / N, tot[2] / N1], np.float32)


def _make_runner(nc):
    """Build the shard_map-jitted executable ONCE (run_bass_kernel_spmd
    re-creates its jit closure every call, which re-ships and re-loads the
    NEFF through the axon tunnel each time)."""
    import jax
    import numpy as _np
    from jax.sharding import Mesh, PartitionSpec
    from jax.experimental.shard_map import shard_map
    import concourse.mybir as mybir
    from concourse import bass2jax
    from concourse.bass2jax import (_bass_exec_p, install_neuronx_cc_hook,
                                    partition_id_tensor)

    install_neuronx_cc_hook()
    pid_name = nc.partition_id_tensor.name if nc.partition_id_tensor else None
    in_names, out_names, out_avals = [], [], []
    for alloc in nc.m.functions[0].allocations:
        if not isinstance(alloc, mybir.MemoryLocationSet):
            continue
        name = alloc.memorylocations[0].name
        if alloc.kind == "ExternalInput":
            if name != pid_name:
                in_names.append(name)
        elif alloc.kind == "ExternalOutput":
            out_names.append(name)
            out_avals.append(jax.core.ShapedArray(
                tuple(alloc.tensor_shape), mybir.dt.np(alloc.dtype)))
    n_params = len(in_names)
    all_names = in_names + out_names
    if pid_name is not None:
        all_names = all_names + [pid_name]
    zero_outs = [_np.zeros(a.shape, a.dtype) for a in out_avals]
    donate = tuple(range(n_params, n_params + len(out_names)))

    def _body(*args):
        operands = list(args)
        if pid_name is not None:
            operands.append(partition_id_tensor())
        outs = _bass_exec_p.bind(
            *operands, out_avals=tuple(out_avals), in_names=tuple(all_names),
            out_names=tuple(out_names), lowering_input_output_aliases=(),
            sim_require_finite=True, sim_require_nnan=True, nc=nc)
        return tuple(outs)

    devices = jax.devices()[:NCORES]
    mesh = Mesh(np.asarray(devices), ("core",))
    in_specs = (PartitionSpec("core"),) * (n_params + len(out_names))
    out_specs = (PartitionSpec("core"),) * len(out_names)
    sharded = jax.jit(
        shard_map(_body, mesh=mesh, in_specs=in_specs, out_specs=out_specs,
                  check_rep=False),
        donate_argnums=donate, keep_unused=True)

    def run(in_maps):
        concat_in = [np.concatenate([m[nm] for m in in_maps], axis=0)
                     for nm in in_names]
        zeros = [np.zeros((NCORES * z.shape[0], *z.shape[1:]), z.dtype)
                 for z in zero_outs]
        out_arrs = sharded(*concat_in, *zeros)
        return [
            {nm: np.asarray(out_arrs[i]).reshape(NCORES, *out_avals[i].shape)[c]
             for i, nm in enumerate(out_names)}
            for c in range(NCORES)
        ]
    return run


def kernel(**inputs) -> np.ndarray:
    if "nc" not in _cached:
        _cached["nc"] = _build_module()
    nc = _cached["nc"]
    in_maps = _host_prep(inputs)
    if "runner" not in _cached:
        try:
            _cached["runner"] = _make_runner(nc)
        except Exception:
            _cached["runner"] = None
    if _cached["runner"] is not None:
        results = _cached["runner"](in_maps)
    else:
        from concourse.bass_utils import run_bass_kernel_spmd
        results = run_bass_kernel_spmd(
            nc, in_maps, core_ids=list(range(NCORES))).results
    return _combine(results)

